# revision 32
# baseline (speedup 1.0000x reference)
"""Trainium2 Bass kernel for nn_DGALoss (SO3 gyro huber losses + sliding-window
velocity normalization loss).

Self-contained: builds one single-core Bass/Tile program, runs it SPMD on 8
NeuronCores (4 batch rows per core), and combines per-core partial sums on the
host.

Math (validated against the jax reference):
- Gyro: product of 16 per-sample rotations exp(DT*w). The first two tree
  levels are replaced by a 2nd-order BCH merge of 4 consecutive small
  rotations (|DT*w| <~ 0.03; truncation ~1e-7 rad typ, far below the fp32
  noise of the reference), then quaternion pair-composition.
- Quaternions are stored component-packed [128, 4, n]; one broadcast-AP
  tensor_tensor computes all 16 cross products, and 4 (reduce, stt) pairs
  assemble the product -> 9 instructions per compose instead of 28.
  Q16/Q32 are stored CONJUGATED (sign flips folded into the combine step;
  composing conj-stored quats = same qmul with operands swapped), so the
  relative rotation D = conj(Q) x P is a plain qmul.
- exp(xs) uses quarter-angle Sin only (ACT Sin range is [-pi,pi]).
- so3_log replicates the reference's clip semantics: c = clip(2*dw^2-1),
  s = sqrt(1-c^2), theta = atan2(s,c) via bounded-arg arctan,
  rs = (2*theta/s)*dw*dv.  The clip CRUSHES rs near theta=pi exactly like
  the reference - do not "fix" it.
- Velocity: the double-cumsum window mean needs only a 16-sample trailing
  window -> per-partition local cumsums (tensor_tensor_scan) over 512-sample
  segments with a 17-sample halo; the global prefix offset cancels in
  vs - mean(window). dv = a[i]+a[i+1] is fused into the first scan's op1.
- ACT table reloads are ~1.4us; Square/Copy/Abs are in every table so only
  sqrt<->trig switches are paid.
"""

import numpy as np

import concourse.bass as bass
import concourse.bacc as bacc
import concourse.mybir as mybir
import concourse.tile as tile
from concourse.bass_types import AP
from concourse.bass_utils import run_bass_kernel_spmd

FP = mybir.dt.float32
BF = mybir.dt.bfloat16
AF = mybir.ActivationFunctionType
OP = mybir.AluOpType

DT = 0.005
HUBER = 0.005
W_LOSS = 1000000.0
N0 = 5
PI = float(np.pi)

B, N, CORES = 32, 65536, 8
ROWS = B // CORES          # 4 batch rows per core
R = 2                      # rows per group
G = ROWS // R              # 2 groups
SEG = N // 128             # 512
M16 = N // 16
M32 = N // 32

QBLK = 96                  # per-group block in QPq/QPp: L16(64)+L32(32)
F16G = 64
F32G = 32
QPF = G * QBLK             # 192 free per comp row

# packed-qmul slot tables (slot = 4*ia + ib in the 16-product tile)
#   w: {0,5,10,15}; x: {1,4,11,14}; y: {2,7,8,13}; z: {3,6,9,12}
# PAIRS[c] = (a0, astep, b0, bstep): slot sum = sum of (a0+i*astep)+(b0+i*bstep)
PAIRS = {0: (0, 5, 15, -5), 1: (1, 3, 14, -3),
         2: (2, 5, 13, -5), 3: (3, 3, 12, -3)}
RED_DIMS = {0: [[5, 4]], 1: [[10, 2], [3, 2]], 2: [[6, 2], [5, 2]], 3: [[3, 4]]}
RED_AX = {0: "X", 1: "XY", 2: "XY", 3: "X"}
NEG_SLOT = {0: 0, 1: 14, 2: 7, 3: 9}   # w's is the lone POSITIVE slot
PAIRWISE_RED = False
FLAT_SCAN = True


def build_kernel(reps=1):
    nc = bacc.Bacc(None)

    w = nc.dram_tensor("w", [3, ROWS, N], BF, kind="ExternalInput")
    a = nc.dram_tensor("a", [3, ROWS, N], BF, kind="ExternalInput")
    gt = nc.dram_tensor("gt", [3, ROWS, N], BF, kind="ExternalInput")
    xs = nc.dram_tensor("xs", [3, 128, 128], FP, kind="ExternalInput")
    stats = nc.dram_tensor("stats", [128, 16], FP, kind="ExternalOutput")

    with tile.TileContext(nc) as tc:
        with (
            tc.tile_pool(name="persist", bufs=1) as pp,
            tc.tile_pool(name="grp_dma", bufs=2) as gd,
            tc.tile_pool(name="grp_tmp", bufs=1) as gtp,
            tc.tile_pool(name="tree", bufs=2) as tp,
            tc.tile_pool(name="small", bufs=2) as sp,
        ):
            for rep_i in range(reps):
                QPq = pp.tile([128, 4 * QPF], FP, name="QPq", tag="QPq")
                QPp = pp.tile([128, 4 * QPF], FP, name="QPp", tag="QPp")
                Q8 = pp.tile([128, 4 * 256], FP, name="Q8", tag="Q8")
                st = pp.tile([128, 16], FP, name="st_t", tag="stats")
                nc.vector.memset(st[:], 0.0)
                pihalf = pp.tile([128, 1], FP, name="pihalf", tag="pihalf")
                nc.vector.memset(pihalf[:], PI / 2.0)
                c16i = pp.tile([128, 1], FP, name="c16i", tag="c16i")
                nc.vector.memset(c16i[:], -1.0 / 16.0)
                cdt = pp.tile([128, 1], FP, name="cdt", tag="cdt")
                nc.vector.memset(cdt[:], -DT)
                c16p = pp.tile([128, 1], BF, name="c16p", tag="c16p")
                nc.vector.memset(c16p[:], 1.0 / 16.0)

                def bcast(t, dims):
                    return AP(tensor=t.tensor, offset=0, ap=[[1, 128]] + dims)

                def comp_ap(t, cf, base, dims):
                    """AP into packed tile t ([128, 4*cf]): comp-row size cf,
                    offset base, extra dims appended."""
                    return AP(tensor=t.tensor, offset=base,
                              ap=[[4 * cf, 128]] + dims)

                def qmul_packed(dst_t, dst_cf, dst_base, a_t, a_cf, a_base,
                                b_t, b_cf, b_base, n, step=1, gdim=None,
                                conj_store=False, eng=None, eng2=None):
                    """dst[comp] = (a x b)[comp] over n lanes (possibly with an
                    extra group dim). a/b read with element stride `step`.
                    gdim: None or (count, a_gstride, b_gstride, dst_gstride).
                    conj_store negates xyz of the result."""
                    eng = eng or nc.vector
                    eng2 = eng2 or eng
                    if gdim is None:
                        gct, ag, bg, dg = 1, 0, 0, 0
                    else:
                        gct, ag, bg, dg = gdim
                    nn = gct * n

                    def gdims(gs):
                        # HW ISA: max 3 free dims; drop the group dim when unit
                        return [] if gct == 1 else [[gs, gct]]

                    P16 = tp.tile([128, 16 * nn], BF, name="P16", tag="P16")
                    a_ap = AP(tensor=a_t.tensor, offset=a_base,
                              ap=[[4 * a_cf, 128], [a_cf, 4], [0, 4]]
                              + gdims(ag) + [[step, n]])
                    b_ap = AP(tensor=b_t.tensor, offset=b_base,
                              ap=[[4 * b_cf, 128], [0, 4], [b_cf, 4]]
                              + gdims(bg) + [[step, n]])
                    o_ap = AP(tensor=P16.tensor, offset=0,
                              ap=[[16 * nn, 128], [4 * nn, 4], [nn, 4]]
                              + gdims(n) + [[1, n]])
                    assert len(a_ap.ap) <= 4, "ISA limit: 3 free dims"
                    eng.tensor_tensor(o_ap, a_ap, b_ap, OP.mult)
                    if PAIRWISE_RED:
                        r2 = tp.tile([128, 2 * nn], BF, name="r2", tag="r2")
                    rtmp = tp.tile([128, nn], BF, name="rtmp", tag="rtmp")
                    for comp in range(4):
                        if PAIRWISE_RED:
                            # 4-slot sum via two pairwise bf16 adds (2x mode)
                            p0, ps, q0, qs = PAIRS[comp]
                            pa = AP(tensor=P16.tensor, offset=p0 * nn,
                                    ap=[[16 * nn, 128], [ps * nn, 2], [1, nn]])
                            pb = AP(tensor=P16.tensor, offset=q0 * nn,
                                    ap=[[16 * nn, 128], [qs * nn, 2], [1, nn]])
                            r2a = AP(tensor=r2.tensor, offset=0,
                                     ap=[[2 * nn, 128], [nn, 2], [1, nn]])
                            nc.vector.tensor_tensor(r2a, pa, pb, OP.add)
                            nc.vector.tensor_add(rtmp[:], r2[:, 0:nn],
                                                 r2[:, nn:])
                        else:
                            dims = [[s * nn, c_] for s, c_ in RED_DIMS[comp]]
                            r_ap = AP(tensor=P16.tensor, offset=comp * nn,
                                      ap=[[16 * nn, 128], [1, nn]] + dims)
                            ax = (mybir.AxisListType.X
                                  if RED_AX[comp] == "X"
                                  else mybir.AxisListType.XY)
                            with nc.allow_low_precision(reason="tol 2e-2"):
                                nc.vector.tensor_reduce(rtmp[:], r_ap, ax,
                                                        OP.add)
                        dst = AP(tensor=dst_t.tensor,
                                 offset=dst_base + comp * dst_cf,
                                 ap=[[4 * dst_cf, 128]] + gdims(dg) + [[1, n]])
                        rt3 = AP(tensor=rtmp.tensor, offset=0,
                                 ap=[[nn, 128]] + gdims(n) + [[1, n]])
                        pneg = AP(tensor=P16.tensor, offset=NEG_SLOT[comp] * nn,
                                  ap=[[16 * nn, 128]] + gdims(n) + [[1, n]])
                        flip = conj_store and comp > 0
                        if comp == 0 or flip:
                            # rw = 2*P[0] - sum; flipped: -(sum - 2*P[neg])
                            eng2.scalar_tensor_tensor(dst, pneg, 2.0, rt3,
                                                      OP.mult, OP.subtract)
                        else:
                            eng2.scalar_tensor_tensor(dst, pneg, -2.0, rt3,
                                                      OP.mult, OP.add)

                # ---------- P = exp(xs), early ----------
                xst = sp.tile([128, 3 * 128], FP, name="xst", tag="xst", bufs=1)
                nc.sync.dma_start(out=xst[:], in_=xs[:, :, :].rearrange(
                    "c p f -> p c f"))
                x3 = xst[:].rearrange("p (c f) -> p c f", c=3)
                sc = [sp.tile([128, 128], FP, name=f"psc{i}", tag=f"psc{i}", bufs=1)
                      for i in range(5)]
                sqx = sp.tile([128, 3 * 128], FP, name="sqx", tag="sqx", bufs=1)
                nc.scalar.activation(sqx[:], xst[:], AF.Square)
                q3 = sqx[:].rearrange("p (c f) -> p c f", c=3)
                nc.vector.tensor_add(sc[0][:], q3[:, 0], q3[:, 1])
                nc.vector.tensor_add(sc[0][:], sc[0][:], q3[:, 2])      # u
                nc.vector.tensor_scalar_max(sc[0][:], sc[0][:], 1e-24)
                nc.scalar.activation(sc[1][:], sc[0][:], AF.Sqrt)       # t
                nc.scalar.activation(sc[2][:], sc[1][:], AF.Sin, scale=0.25)  # s4
                nc.scalar.activation(sc[3][:], sc[1][:], AF.Sin, scale=-0.25,
                                     bias=pihalf[:, 0:1])               # c4
                nc.vector.scalar_tensor_tensor(sc[4][:], sc[2][:], 2.0, sc[3][:],
                                               OP.mult, OP.mult)        # s2
                nc.vector.scalar_tensor_tensor(sc[2][:], sc[2][:], -2.0, sc[2][:],
                                               OP.mult, OP.mult)        # -2 s4^2
                nc.vector.reciprocal(sc[1][:], sc[1][:])
                nc.vector.tensor_mul(sc[4][:], sc[4][:], sc[1][:])      # coef
                # pw -> QPp comp0 blocks; pv -> comps 1..3  (g f) = (row pair, 32)
                pw_dst = AP(tensor=QPp.tensor, offset=0,
                            ap=[[4 * QPF, 128], [QBLK, G], [1, F16G]])
                nc.vector.tensor_scalar_add(
                    pw_dst, sc[2][:].rearrange("p (g f) -> p g f", g=G), 1.0)
                pv_dst = AP(tensor=QPp.tensor, offset=QPF,
                            ap=[[4 * QPF, 128], [QPF, 3], [QBLK, G], [1, F16G]])
                cfb = AP(tensor=sc[4].tensor, offset=0,
                         ap=[[128, 128], [0, 3], [64, G], [1, F16G]])
                xv = AP(tensor=xst.tensor, offset=0,
                        ap=[[3 * 128, 128], [128, 3], [64, G], [1, F16G]])
                nc.vector.tensor_tensor(pv_dst, cfb, xv, OP.mult)

                # level-4 P-part: P pairs -> P32 (both groups, normal store)
                for gg in range(G):
                    qmul_packed(QPp, QPF, F16G + gg * QBLK,
                                QPp, QPF, gg * QBLK, QPp, QPF, gg * QBLK + 1,
                                F32G, step=2, eng=nc.vector, eng2=nc.vector)

                # ---------- per-group ----------
                for g in range(G):
                    rows = slice(g * R, (g + 1) * R)
                    F8 = R * SEG // 8              # 128
                    CF = R * SEG                   # comp row size in Wd (1024)
                    # Wd: comps x,y,z packed [128, 3, 1024]
                    Wd = gd.tile([128, 3 * CF], BF, name="Wd", tag="Wd")
                    w3 = Wd[:].rearrange("p (c f) -> p c f", c=3)
                    for c in range(3):
                        nc.sync.dma_start(
                            out=w3[:, c].rearrange("p (r j) -> p r j", j=SEG),
                            in_=w[c, rows, :].rearrange("r (p j) -> p r j", j=SEG),
                        )
                    # de-interleave sample phases once on ACT: Wp[c,k,f] =
                    # Wd[c, 8f+k] -> all BCH reads below are stride-1 (DVE 2x)
                    NT = 7
                    KF = 8 * F8
                    Wp = gtp.tile([128, 3 * KF], BF, name="Wp", tag="Wp")
                    nc.scalar.activation(
                        AP(tensor=Wp.tensor, offset=0,
                           ap=[[3 * KF, 128], [KF, 3], [F8, 8], [1, F8]]),
                        AP(tensor=Wd.tensor, offset=0,
                           ap=[[3 * CF, 128], [CF, 3], [1, 8], [8, F8]]),
                        AF.Copy)
                    # BCH-8: prefixes PP [128, 3, 7, 128] = P0..P6 per comp
                    PP = gtp.tile([128, 3 * NT * F8], BF, name="PP", tag="PP")
                    wk = lambda k, c0, cn, cs=1: AP(
                        tensor=Wp.tensor, offset=c0 * KF + k * F8,
                        ap=[[3 * KF, 128], [cs * KF, cn], [1, F8]])
                    ppv = lambda t_, c0, cn, cs=1: AP(
                        tensor=PP.tensor, offset=(c0 * NT + t_) * F8,
                        ap=[[3 * NT * F8, 128], [cs * NT * F8, cn], [1, F8]])
                    nc.scalar.activation(ppv(0, 0, 3), wk(0, 0, 3), AF.Copy)
                    for t_ in range(1, NT):
                        nc.vector.tensor_tensor(ppv(t_, 0, 3), ppv(t_ - 1, 0, 3),
                                                wk(t_, 0, 3), OP.add)
                    S_t = gtp.tile([128, 3 * F8], BF, name="S_t", tag="S_t")
                    s3 = S_t[:].rearrange("p (c f) -> p c f", c=3)
                    nc.vector.tensor_tensor(s3[:], ppv(NT - 1, 0, 3),
                                            wk(NT, 0, 3), OP.add)
                    # crosses: m1[c,t] = PP[(c+1)%3, t] * w[(c+2)%3, t+1]
                    #          m2[c,t] = PP[(c+2)%3, t] * w[(c+1)%3, t+1]
                    m1 = gtp.tile([128, 3 * NT * F8], BF, name="m1", tag="m1")
                    m2 = gtp.tile([128, 3 * NT * F8], BF, name="m2", tag="m2")
                    wsh = lambda c0, cn, cs=1: AP(
                        tensor=Wp.tensor, offset=c0 * KF + F8,
                        ap=[[3 * KF, 128], [cs * KF, cn], [F8, NT], [1, F8]])
                    ppt = lambda c0, cn, cs=1: AP(
                        tensor=PP.tensor, offset=c0 * NT * F8,
                        ap=[[3 * NT * F8, 128], [cs * NT * F8, cn], [F8, NT],
                            [1, F8]])
                    mv1 = m1[:].rearrange("p (c t f) -> p c t f", c=3, t=NT)
                    mv2 = m2[:].rearrange("p (c t f) -> p c t f", c=3, t=NT)
                    nc.vector.tensor_tensor(mv1[:, 0:1], ppt(1, 1), wsh(2, 1),
                                            OP.mult)
                    nc.vector.tensor_tensor(mv1[:, 1:3], ppt(2, 2, -2),
                                            wsh(0, 2), OP.mult)
                    nc.vector.tensor_tensor(mv2[:, 0:1], ppt(2, 1), wsh(1, 1),
                                            OP.mult)
                    nc.vector.tensor_tensor(mv2[:, 1:3], ppt(0, 2), wsh(2, 2, -2),
                                            OP.mult)
                    nc.vector.tensor_sub(m1[:], m1[:], m2[:])
                    # csum over the 7 terms: pairwise bf16 adds (2x mode) beat
                    # a free-axis tensor_reduce (no fast mode); m2 is scratch
                    Ct = gtp.tile([128, 3 * F8], BF, name="Ct", tag="Ct")
                    c3 = Ct[:].rearrange("p (c f) -> p c f", c=3)
                    nc.vector.tensor_add(mv2[:, :, 0:3], mv1[:, :, 0:3],
                                         mv1[:, :, 3:6])
                    nc.vector.tensor_add(mv2[:, :, 3:4], mv2[:, :, 0:1],
                                         mv2[:, :, 1:2])
                    nc.vector.tensor_add(mv2[:, :, 4:5], mv2[:, :, 2:3],
                                         mv1[:, :, 6:7])
                    nc.vector.tensor_add(c3[:], mv2[:, :, 3], mv2[:, :, 4])
                    # phi' = S + (DT/2) C   (true phi = DT*phi')
                    nc.vector.scalar_tensor_tensor(c3[:], c3[:], DT / 2.0, s3[:],
                                                   OP.mult, OP.add)

                    # exp_small: u = |phi'|^2 -> Q8 group block (packed)
                    squ = gtp.tile([128, 3 * F8], FP, name="squ", tag="squ")
                    nc.scalar.activation(squ[:], Ct[:], AF.Square)
                    z3 = squ[:].rearrange("p (c f) -> p c f", c=3)
                    ut = gtp.tile([128, F8], FP, name="ut", tag="ut")
                    nc.vector.tensor_add(ut[:], z3[:, 0], z3[:, 1])
                    nc.vector.tensor_add(ut[:], ut[:], z3[:, 2])
                    qs = gtp.tile([128, F8], FP, name="qs_t", tag="qs")
                    gq = g * F8
                    q83 = Q8[:].rearrange("p (c f) -> p c f", c=4)
                    nc.scalar.activation(q83[:, 0, gq:gq + F8], ut[:], AF.Copy,
                                         bias=1.0, scale=-(DT * DT) / 8.0)
                    nc.scalar.activation(qs[:], ut[:], AF.Copy,
                                         bias=DT / 2.0, scale=-(DT ** 3) / 48.0)
                    qv_dst = AP(tensor=Q8.tensor, offset=256 + gq,
                                ap=[[4 * 256, 128], [256, 3], [1, F8]])
                    qsb = AP(tensor=qs.tensor, offset=0,
                             ap=[[F8, 128], [0, 3], [1, F8]])
                    nc.vector.tensor_tensor(qv_dst, qsb, c3[:], OP.mult)

                    # tree: Q8 -> Q16* (conj store)
                    qb = g * QBLK
                    qmul_packed(QPq, QPF, qb, Q8, 256, g * 128,
                                Q8, 256, g * 128 + 1, F16G, step=2,
                                conj_store=True, eng=nc.vector, eng2=nc.vector)
                    # level-4 Q: Q32* = qmul(Q16*_odd, Q16*_even)  (conj swap)
                    qmul_packed(QPq, QPF, qb + F16G, QPq, QPF, qb + 1,
                                QPq, QPF, qb, F32G, step=2,
                                eng=nc.gpsimd, eng2=nc.vector)

                    # -- velocity
                    HA = SEG + 17
                    HD = SEG + 16
                    gtt = gd.tile([128, 3 * R * SEG], BF, name="gtt", tag="gtt", bufs=1)
                    gt3 = gtt[:].rearrange("p (c r j) -> p c r j", c=3, r=R)
                    NB = 3 * R                     # 6 (c,r) blocks
                    FLT = NB * HA                  # 3174 flat scan length
                    aht = gd.tile([128, FLT + 1], BF, name="aht", tag="aht")
                    ah3 = aht[:, 0:FLT].rearrange("p (c r u) -> p c r u",
                                                  c=3, r=R)
                    for c in range(3):
                        nc.sync.dma_start(
                            out=gt3[:, c],
                            in_=gt[c, rows, :].rearrange("r (p j) -> p r j", j=SEG))
                        src = AP(tensor=a[:].tensor,
                                 offset=c * ROWS * N + g * R * N + 495,
                                 ap=[[SEG, 127], [N, R], [1, HA]])
                        nc.sync.dma_start(out=ah3[1:128, c], in_=src)
                        nc.sync.dma_start(out=ah3[0:1, c, :, 17:HA],
                                          in_=a[c, rows, 0:SEG])
                    nc.gpsimd.memset(ah3[0:1, :, :, 0:17], 0.0)
                    nc.gpsimd.memset(aht[:, FLT:FLT + 1], 0.0)
                    s1 = gtp.tile([128, FLT], BF, name="s1_t", tag="s1")
                    s2 = gtp.tile([128, FLT], BF, name="s2_t", tag="s2")
                    # a is host-prescaled by DT (s1 = DT*vs directly) and the
                    # 1/16 window mean folds into scan2's op1 -> every window
                    # op below is a plain tensor_sub (DVE 2x bf16 mode).
                    # ONE flat scan per stage over all (c,r) blocks: the
                    # carry across block boundaries (incl. the 1-elem gap) is
                    # constant per block and cancels in vs - mean(window)
                    # (same cancellation the 17-sample halo relies on); scan
                    # state is fp32 internally.
                    c16b = AP(tensor=c16p.tensor, offset=0,
                              ap=[[1, 128], [0, FLT]])
                    nc.vector.tensor_tensor_scan(
                        s1[:], aht[:, 0:FLT], aht[:, 1:FLT + 1], 0.0,
                        OP.add, OP.add)
                    nc.vector.tensor_tensor_scan(
                        s2[:], s1[:], c16b, 0.0, OP.add, OP.mult)
                    t1 = gtp.tile([128, NB * SEG], BF, name="t1_t", tag="t1")
                    blk = lambda t, off, bs=HA: AP(
                        tensor=t.tensor, offset=off,
                        ap=[[t.shape[1], 128], [bs, NB], [1, SEG]])
                    t13a = blk(t1, 0, bs=SEG)
                    gta = blk(gtt, 0, bs=SEG)
                    # same in-block window offsets as the per-(c,r) version:
                    # [16:528] vs [0:512]; block stride is HA=529 (the 529th
                    # slot is the flat-scan gap element, unused here)
                    nc.vector.tensor_tensor(t13a, blk(s2, 16), blk(s2, 0),
                                            OP.subtract)
                    nc.vector.tensor_tensor(blk(s2, 0), blk(s1, 16), t13a,
                                            OP.subtract)          # vsn
                    nc.vector.tensor_tensor(blk(s1, 0), gta, blk(s2, 0),
                                            OP.subtract)          # err
                    nc.scalar.activation(gta, blk(s1, 0), AF.Square,
                                         accum_out=st[:, 4 + g:5 + g])

                # ---------- merged D + log + huber (both groups, [128, 192]) ----
                if True:
                    QB2 = G * QBLK                 # 192: both groups contiguous
                    # D = Qstar x P for BOTH groups in one packed qmul
                    Dp = sp.tile([128, 4 * QB2], FP, name="Dp", tag="Dp")
                    qmul_packed(Dp, QB2, 0, QPq, QPF, 0, QPp, QPF, 0,
                                QB2, step=1, eng=nc.gpsimd, eng2=nc.vector)
                    d4 = Dp[:].rearrange("p (c f) -> p c f", c=4)

                    # log + huber on [128, 192]
                    l0 = [sp.tile([128, QB2], FP, name=f"lg{i}", tag=f"lg{i}")
                          for i in range(6)]
                    cm = sp.tile([128, QB2], mybir.dt.int32, name="cmask",
                                 tag="cmask")
                    nc.scalar.activation(l0[0][:], d4[:, 0], AF.Square)      # dw^2
                    nc.vector.tensor_scalar(l0[1][:], l0[0][:], 2.0, -1.0,
                                            OP.mult, OP.add)
                    nc.vector.tensor_scalar(l0[1][:], l0[1][:], 1.0 - 1e-7,
                                            -1.0 + 1e-7, OP.min, OP.max)     # c
                    nc.scalar.activation(l0[0][:], l0[1][:], AF.Square)
                    nc.scalar.activation(l0[2][:], l0[0][:], AF.Sqrt,
                                         bias=1.0, scale=-1.0)               # s
                    nc.scalar.activation(l0[3][:], l0[1][:], AF.Abs)         # |c|
                    nc.vector.tensor_tensor(l0[4][:], l0[2][:], l0[3][:], OP.min)
                    nc.vector.tensor_tensor(l0[5][:], l0[2][:], l0[3][:], OP.max)
                    nc.vector.reciprocal(l0[5][:], l0[5][:])
                    nc.vector.tensor_mul(l0[4][:], l0[4][:], l0[5][:])
                    nc.scalar.activation(l0[4][:], l0[4][:], AF.Arctan)      # atn
                    nc.vector.tensor_tensor(cm[:], l0[3][:], l0[2][:], OP.is_ge)
                    nc.scalar.activation(l0[5][:], l0[4][:], AF.Copy, scale=-1.0,
                                         bias=PI / 2.0)
                    nc.vector.copy_predicated(l0[5][:], cm[:], l0[4][:])     # th0
                    nc.vector.tensor_scalar(cm[:], l0[1][:], 0.0, None, OP.is_ge)
                    nc.scalar.activation(l0[3][:], l0[5][:], AF.Copy, scale=-1.0,
                                         bias=PI)
                    nc.vector.copy_predicated(l0[3][:], cm[:], l0[5][:])     # theta
                    nc.vector.reciprocal(l0[2][:], l0[2][:])
                    nc.vector.tensor_mul(l0[3][:], l0[3][:], l0[2][:])
                    nc.vector.scalar_tensor_tensor(l0[3][:], l0[3][:], 2.0,
                                                   d4[:, 0], OP.mult, OP.mult)
                    # huber (merged comps): rs = coef*dv
                    rsv = sp.tile([128, 3 * QB2], FP, name="rsv", tag="rsv")
                    r3 = rsv[:].rearrange("p (c f) -> p c f", c=3)
                    cfb2 = AP(tensor=l0[3].tensor, offset=0,
                              ap=[[QB2, 128], [0, 3], [1, QB2]])
                    nc.vector.tensor_tensor(r3[:], cfb2, d4[:, 1:4], OP.mult)
                    axv = sp.tile([128, 3 * QB2], FP, name="axv", tag="axv")
                    nc.scalar.activation(axv[:], rsv[:], AF.Abs, scale=1.0 / HUBER)
                    mv = sp.tile([128, 3 * QB2], FP, name="mv", tag="mv")
                    nc.vector.tensor_scalar_min(mv[:], axv[:], 1.0)
                    t5 = sp.tile([128, 3 * QB2], FP, name="t5", tag="t5")
                    nc.vector.scalar_tensor_tensor(t5[:], mv[:], -1.0, axv[:],
                                                   OP.mult, OP.add)
                    nc.vector.scalar_tensor_tensor(mv[:], mv[:], 0.5, mv[:],
                                                   OP.mult, OP.mult)
                    nc.gpsimd.tensor_add(t5[:], t5[:], mv[:])                # l
                    lt = t5[:].rearrange("p (c f) -> p c f", c=3)
                    lsum = sp.tile([128, QB2], FP, name="lsum", tag="lsum")
                    nc.gpsimd.tensor_add(lsum[:], lt[:, 0], lt[:, 1])
                    nc.gpsimd.tensor_add(lsum[:], lsum[:], lt[:, 2])
                    for base, width in ((0, F16G), (F16G, F32G),
                                        (QBLK, F16G), (QBLK + F16G, F32G)):
                        nc.vector.memset(
                            lsum[0:1, base:base + width].rearrange(
                                "p (row j) -> p row j",
                                j=width // R)[:, :, 0:N0], 0.0)
                    for col, base, width in ((1, 0, F16G), (2, F16G, F32G),
                                             (11, QBLK, F16G),
                                             (12, QBLK + F16G, F32G)):
                        nc.vector.tensor_reduce(st[:, col:col + 1],
                                                lsum[:, base:base + width],
                                                mybir.AxisListType.X, OP.add)

                nc.sync.dma_start(out=stats[:], in_=st[:])

    nc.compile()
    return nc


_NC = None


def _get_nc():
    global _NC
    if _NC is None:
        _NC = build_kernel()
    return _NC


def shard_inputs(w_hat, a_hat, xs, dv, vs_gt_norm):
    """Full inputs -> per-core input maps (planar component-major layout).

    w/a/gt are fed to the device as bf16 (host-side cast): halves DMA and
    unlocks the DVE 2x/4x perf modes; the loss tolerance (2e-2) dwarfs the
    bf16 rounding (see docstring)."""
    del dv  # unused by the reference computation
    import concourse.mybir as _mb
    bf = _mb.dt.np(_mb.dt.bfloat16)
    in_maps = []
    for core in range(CORES):
        rows = slice(core * ROWS, (core + 1) * ROWS)
        xsub = xs[rows, ::16]                      # [ROWS, M16, 3]
        xdev = xsub.reshape(ROWS, 128, M16 // 128, 3).transpose(3, 1, 0, 2)
        in_maps.append({
            "w": np.ascontiguousarray(w_hat[rows].transpose(2, 0, 1)).astype(bf),
            "a": (np.ascontiguousarray(a_hat[rows].transpose(2, 0, 1))
                  * np.float32(DT)).astype(bf),
            "gt": np.ascontiguousarray(vs_gt_norm[rows].transpose(2, 0, 1)).astype(bf),
            "xs": np.ascontiguousarray(xdev.reshape(3, 128, 128)),
        })
    return in_maps


def combine_stats(stats_list):
    """Per-core [128,16] partials -> final scalar loss (fp64 host combine)."""
    s = np.sum([st.astype(np.float64) for st in stats_list], axis=(0, 1))
    sq_total = float(np.sum(s[4:10]))
    l16 = float(s[1] + s[11])
    l32 = float(s[2] + s[12])
    acc = sq_total / (B * N * 3)
    g16 = W_LOSS * HUBER * HUBER * l16 / (B * (M16 - N0) * 3)
    g32 = W_LOSS * HUBER * HUBER * l32 / (B * (M32 - N0) * 3) / 2.0
    return np.float32(g16 + g32 + acc)


def kernel(**inputs):
    nc = _get_nc()
    in_maps = shard_inputs(**inputs)
    res = run_bass_kernel_spmd(nc, in_maps, list(range(CORES)))
    return combine_stats([r["stats"] for r in res.results])



# revision 33
# speedup vs baseline: 1.6226x; 1.6226x over previous
"""Trainium2 Bass kernel for nn_DGALoss (SO3 gyro huber losses + sliding-window
velocity normalization loss).

Self-contained: builds one single-core Bass/Tile program, runs it SPMD on 8
NeuronCores (4 batch rows per core), and combines per-core partial sums on the
host.

Math (validated against the jax reference):
- Gyro: product of 16 per-sample rotations exp(DT*w). The first two tree
  levels are replaced by a 2nd-order BCH merge of 4 consecutive small
  rotations (|DT*w| <~ 0.03; truncation ~1e-7 rad typ, far below the fp32
  noise of the reference), then quaternion pair-composition.
- Quaternions are stored component-packed [128, 4, n]; one broadcast-AP
  tensor_tensor computes all 16 cross products, and 4 (reduce, stt) pairs
  assemble the product -> 9 instructions per compose instead of 28.
  Q16/Q32 are stored CONJUGATED (sign flips folded into the combine step;
  composing conj-stored quats = same qmul with operands swapped), so the
  relative rotation D = conj(Q) x P is a plain qmul.
- exp(xs) uses quarter-angle Sin only (ACT Sin range is [-pi,pi]).
- so3_log replicates the reference's clip semantics: c = clip(2*dw^2-1),
  s = sqrt(1-c^2), theta = atan2(s,c) via bounded-arg arctan,
  rs = (2*theta/s)*dw*dv.  The clip CRUSHES rs near theta=pi exactly like
  the reference - do not "fix" it.
- Velocity: the double-cumsum window mean needs only a 16-sample trailing
  window -> per-partition local cumsums (tensor_tensor_scan) over 512-sample
  segments with a 17-sample halo; the global prefix offset cancels in
  vs - mean(window). dv = a[i]+a[i+1] is fused into the first scan's op1.
- ACT table reloads are ~1.4us; Square/Copy/Abs are in every table so only
  sqrt<->trig switches are paid.
"""

import numpy as np

import concourse.bass as bass
import concourse.bacc as bacc
import concourse.mybir as mybir
import concourse.tile as tile
from concourse.bass_types import AP
from concourse.bass_utils import run_bass_kernel_spmd

FP = mybir.dt.float32
BF = mybir.dt.bfloat16
AF = mybir.ActivationFunctionType
OP = mybir.AluOpType

DT = 0.005
HUBER = 0.005
W_LOSS = 1000000.0
N0 = 5
PI = float(np.pi)

B, N, CORES = 32, 65536, 8
ROWS = B // CORES          # 4 batch rows per core
R = 2                      # rows per group
G = ROWS // R              # 2 groups
SEG = N // 128             # 512
M16 = N // 16
M32 = N // 32

QBLK = 96                  # per-group block in QPq/QPp: L16(64)+L32(32)
F16G = 64
F32G = 32
QPF = G * QBLK             # 192 free per comp row

# packed-qmul slot tables (slot = 4*ia + ib in the 16-product tile)
#   w: {0,5,10,15}; x: {1,4,11,14}; y: {2,7,8,13}; z: {3,6,9,12}
# PAIRS[c] = (a0, astep, b0, bstep): slot sum = sum of (a0+i*astep)+(b0+i*bstep)
PAIRS = {0: (0, 5, 15, -5), 1: (1, 3, 14, -3),
         2: (2, 5, 13, -5), 3: (3, 3, 12, -3)}
RED_DIMS = {0: [[5, 4]], 1: [[10, 2], [3, 2]], 2: [[6, 2], [5, 2]], 3: [[3, 4]]}
RED_AX = {0: "X", 1: "XY", 2: "XY", 3: "X"}
NEG_SLOT = {0: 0, 1: 14, 2: 7, 3: 9}   # w's is the lone POSITIVE slot
PAIRWISE_RED = False
FLAT_SCAN = True


def build_kernel(reps=1):
    nc = bacc.Bacc(None)

    w = nc.dram_tensor("w", [3, ROWS, N], BF, kind="ExternalInput")
    a = nc.dram_tensor("a", [3, ROWS, N], BF, kind="ExternalInput")
    gt = nc.dram_tensor("gt", [3, ROWS, N], BF, kind="ExternalInput")
    xs = nc.dram_tensor("xs", [3, 128, 128], FP, kind="ExternalInput")
    stats = nc.dram_tensor("stats", [128, 16], FP, kind="ExternalOutput")

    with tile.TileContext(nc) as tc:
        with (
            tc.tile_pool(name="persist", bufs=1) as pp,
            tc.tile_pool(name="grp_dma", bufs=2) as gd,
            tc.tile_pool(name="grp_tmp", bufs=1) as gtp,
            tc.tile_pool(name="tree", bufs=2) as tp,
            tc.tile_pool(name="small", bufs=2) as sp,
        ):
            for rep_i in range(reps):
                QPq = pp.tile([128, 4 * QPF], FP, name="QPq", tag="QPq")
                QPp = pp.tile([128, 4 * QPF], FP, name="QPp", tag="QPp")
                Q8 = pp.tile([128, 4 * 256], FP, name="Q8", tag="Q8")
                st = pp.tile([128, 16], FP, name="st_t", tag="stats")
                nc.vector.memset(st[:], 0.0)
                pihalf = pp.tile([128, 1], FP, name="pihalf", tag="pihalf")
                nc.vector.memset(pihalf[:], PI / 2.0)
                c16i = pp.tile([128, 1], FP, name="c16i", tag="c16i")
                nc.vector.memset(c16i[:], -1.0 / 16.0)
                cdt = pp.tile([128, 1], FP, name="cdt", tag="cdt")
                nc.vector.memset(cdt[:], -DT)
                c16p = pp.tile([128, 1], BF, name="c16p", tag="c16p")
                nc.vector.memset(c16p[:], 1.0 / 16.0)

                def bcast(t, dims):
                    return AP(tensor=t.tensor, offset=0, ap=[[1, 128]] + dims)

                def comp_ap(t, cf, base, dims):
                    """AP into packed tile t ([128, 4*cf]): comp-row size cf,
                    offset base, extra dims appended."""
                    return AP(tensor=t.tensor, offset=base,
                              ap=[[4 * cf, 128]] + dims)

                def qmul_packed(dst_t, dst_cf, dst_base, a_t, a_cf, a_base,
                                b_t, b_cf, b_base, n, step=1, gdim=None,
                                conj_store=False, eng=None, eng2=None):
                    """dst[comp] = (a x b)[comp] over n lanes (possibly with an
                    extra group dim). a/b read with element stride `step`.
                    gdim: None or (count, a_gstride, b_gstride, dst_gstride).
                    conj_store negates xyz of the result."""
                    eng = eng or nc.vector
                    eng2 = eng2 or eng
                    if gdim is None:
                        gct, ag, bg, dg = 1, 0, 0, 0
                    else:
                        gct, ag, bg, dg = gdim
                    nn = gct * n

                    def gdims(gs):
                        # HW ISA: max 3 free dims; drop the group dim when unit
                        return [] if gct == 1 else [[gs, gct]]

                    P16 = tp.tile([128, 16 * nn], BF, name="P16", tag="P16")
                    a_ap = AP(tensor=a_t.tensor, offset=a_base,
                              ap=[[4 * a_cf, 128], [a_cf, 4], [0, 4]]
                              + gdims(ag) + [[step, n]])
                    b_ap = AP(tensor=b_t.tensor, offset=b_base,
                              ap=[[4 * b_cf, 128], [0, 4], [b_cf, 4]]
                              + gdims(bg) + [[step, n]])
                    o_ap = AP(tensor=P16.tensor, offset=0,
                              ap=[[16 * nn, 128], [4 * nn, 4], [nn, 4]]
                              + gdims(n) + [[1, n]])
                    assert len(a_ap.ap) <= 4, "ISA limit: 3 free dims"
                    eng.tensor_tensor(o_ap, a_ap, b_ap, OP.mult)
                    if PAIRWISE_RED:
                        r2 = tp.tile([128, 2 * nn], BF, name="r2", tag="r2")
                    rtmp = tp.tile([128, nn], BF, name="rtmp", tag="rtmp")
                    for comp in range(4):
                        if PAIRWISE_RED:
                            # 4-slot sum via two pairwise bf16 adds (2x mode)
                            p0, ps, q0, qs = PAIRS[comp]
                            pa = AP(tensor=P16.tensor, offset=p0 * nn,
                                    ap=[[16 * nn, 128], [ps * nn, 2], [1, nn]])
                            pb = AP(tensor=P16.tensor, offset=q0 * nn,
                                    ap=[[16 * nn, 128], [qs * nn, 2], [1, nn]])
                            r2a = AP(tensor=r2.tensor, offset=0,
                                     ap=[[2 * nn, 128], [nn, 2], [1, nn]])
                            nc.vector.tensor_tensor(r2a, pa, pb, OP.add)
                            nc.vector.tensor_add(rtmp[:], r2[:, 0:nn],
                                                 r2[:, nn:])
                        else:
                            dims = [[s * nn, c_] for s, c_ in RED_DIMS[comp]]
                            r_ap = AP(tensor=P16.tensor, offset=comp * nn,
                                      ap=[[16 * nn, 128], [1, nn]] + dims)
                            ax = (mybir.AxisListType.X
                                  if RED_AX[comp] == "X"
                                  else mybir.AxisListType.XY)
                            with nc.allow_low_precision(reason="tol 2e-2"):
                                nc.vector.tensor_reduce(rtmp[:], r_ap, ax,
                                                        OP.add)
                        dst = AP(tensor=dst_t.tensor,
                                 offset=dst_base + comp * dst_cf,
                                 ap=[[4 * dst_cf, 128]] + gdims(dg) + [[1, n]])
                        rt3 = AP(tensor=rtmp.tensor, offset=0,
                                 ap=[[nn, 128]] + gdims(n) + [[1, n]])
                        pneg = AP(tensor=P16.tensor, offset=NEG_SLOT[comp] * nn,
                                  ap=[[16 * nn, 128]] + gdims(n) + [[1, n]])
                        flip = conj_store and comp > 0
                        if comp == 0 or flip:
                            # rw = 2*P[0] - sum; flipped: -(sum - 2*P[neg])
                            eng2.scalar_tensor_tensor(dst, pneg, 2.0, rt3,
                                                      OP.mult, OP.subtract)
                        else:
                            eng2.scalar_tensor_tensor(dst, pneg, -2.0, rt3,
                                                      OP.mult, OP.add)

                # ---------- P = exp(xs), early ----------
                xst = sp.tile([128, 3 * 128], FP, name="xst", tag="xst", bufs=1)
                nc.sync.dma_start(out=xst[:], in_=xs[:, :, :].rearrange(
                    "c p f -> p c f"))
                x3 = xst[:].rearrange("p (c f) -> p c f", c=3)
                sc = [sp.tile([128, 128], FP, name=f"psc{i}", tag=f"psc{i}", bufs=1)
                      for i in range(5)]
                sqx = sp.tile([128, 3 * 128], FP, name="sqx", tag="sqx", bufs=1)
                nc.scalar.activation(sqx[:], xst[:], AF.Square)
                q3 = sqx[:].rearrange("p (c f) -> p c f", c=3)
                nc.vector.tensor_add(sc[0][:], q3[:, 0], q3[:, 1])
                nc.vector.tensor_add(sc[0][:], sc[0][:], q3[:, 2])      # u
                nc.vector.tensor_scalar_max(sc[0][:], sc[0][:], 1e-24)
                nc.scalar.activation(sc[1][:], sc[0][:], AF.Sqrt)       # t
                nc.scalar.activation(sc[2][:], sc[1][:], AF.Sin, scale=0.25)  # s4
                nc.scalar.activation(sc[3][:], sc[1][:], AF.Sin, scale=-0.25,
                                     bias=pihalf[:, 0:1])               # c4
                nc.vector.scalar_tensor_tensor(sc[4][:], sc[2][:], 2.0, sc[3][:],
                                               OP.mult, OP.mult)        # s2
                nc.vector.scalar_tensor_tensor(sc[2][:], sc[2][:], -2.0, sc[2][:],
                                               OP.mult, OP.mult)        # -2 s4^2
                nc.vector.reciprocal(sc[1][:], sc[1][:])
                nc.vector.tensor_mul(sc[4][:], sc[4][:], sc[1][:])      # coef
                # pw -> QPp comp0 blocks; pv -> comps 1..3  (g f) = (row pair, 32)
                pw_dst = AP(tensor=QPp.tensor, offset=0,
                            ap=[[4 * QPF, 128], [QBLK, G], [1, F16G]])
                nc.vector.tensor_scalar_add(
                    pw_dst, sc[2][:].rearrange("p (g f) -> p g f", g=G), 1.0)
                pv_dst = AP(tensor=QPp.tensor, offset=QPF,
                            ap=[[4 * QPF, 128], [QPF, 3], [QBLK, G], [1, F16G]])
                cfb = AP(tensor=sc[4].tensor, offset=0,
                         ap=[[128, 128], [0, 3], [64, G], [1, F16G]])
                xv = AP(tensor=xst.tensor, offset=0,
                        ap=[[3 * 128, 128], [128, 3], [64, G], [1, F16G]])
                nc.vector.tensor_tensor(pv_dst, cfb, xv, OP.mult)

                # level-4 P-part: P pairs -> P32 (both groups, normal store)
                for gg in range(G):
                    qmul_packed(QPp, QPF, F16G + gg * QBLK,
                                QPp, QPF, gg * QBLK, QPp, QPF, gg * QBLK + 1,
                                F32G, step=2, eng=nc.vector, eng2=nc.vector)

                # ---------- per-group ----------
                for g in range(G):
                    rows = slice(g * R, (g + 1) * R)
                    F8 = R * SEG // 8              # 128
                    CF = R * SEG                   # comp row size in Wd (1024)
                    # Wd: comps x,y,z packed [128, 3, 1024]
                    Wd = gd.tile([128, 3 * CF], BF, name="Wd", tag="Wd")
                    w3 = Wd[:].rearrange("p (c f) -> p c f", c=3)
                    for c in range(3):
                        nc.sync.dma_start(
                            out=w3[:, c].rearrange("p (r j) -> p r j", j=SEG),
                            in_=w[c, rows, :].rearrange("r (p j) -> p r j", j=SEG),
                        )
                    # de-interleave sample phases once on ACT: Wp[c,k,f] =
                    # Wd[c, 8f+k] -> all BCH reads below are stride-1 (DVE 2x)
                    NT = 7
                    KF = 8 * F8
                    Wp = gtp.tile([128, 3 * KF], BF, name="Wp", tag="Wp")
                    nc.scalar.activation(
                        AP(tensor=Wp.tensor, offset=0,
                           ap=[[3 * KF, 128], [KF, 3], [F8, 8], [1, F8]]),
                        AP(tensor=Wd.tensor, offset=0,
                           ap=[[3 * CF, 128], [CF, 3], [1, 8], [8, F8]]),
                        AF.Copy)
                    # BCH-8: prefixes PP [128, 3, 7, 128] = P0..P6 per comp
                    PP = gtp.tile([128, 3 * NT * F8], BF, name="PP", tag="PP")
                    wk = lambda k, c0, cn, cs=1: AP(
                        tensor=Wp.tensor, offset=c0 * KF + k * F8,
                        ap=[[3 * KF, 128], [cs * KF, cn], [1, F8]])
                    ppv = lambda t_, c0, cn, cs=1: AP(
                        tensor=PP.tensor, offset=(c0 * NT + t_) * F8,
                        ap=[[3 * NT * F8, 128], [cs * NT * F8, cn], [1, F8]])
                    nc.scalar.activation(ppv(0, 0, 3), wk(0, 0, 3), AF.Copy)
                    for t_ in range(1, NT):
                        nc.vector.tensor_tensor(ppv(t_, 0, 3), ppv(t_ - 1, 0, 3),
                                                wk(t_, 0, 3), OP.add)
                    S_t = gtp.tile([128, 3 * F8], BF, name="S_t", tag="S_t")
                    s3 = S_t[:].rearrange("p (c f) -> p c f", c=3)
                    nc.vector.tensor_tensor(s3[:], ppv(NT - 1, 0, 3),
                                            wk(NT, 0, 3), OP.add)
                    # crosses: m1[c,t] = PP[(c+1)%3, t] * w[(c+2)%3, t+1]
                    #          m2[c,t] = PP[(c+2)%3, t] * w[(c+1)%3, t+1]
                    m1 = gtp.tile([128, 3 * NT * F8], BF, name="m1", tag="m1")
                    m2 = gtp.tile([128, 3 * NT * F8], BF, name="m2", tag="m2")
                    wsh = lambda c0, cn, cs=1: AP(
                        tensor=Wp.tensor, offset=c0 * KF + F8,
                        ap=[[3 * KF, 128], [cs * KF, cn], [F8, NT], [1, F8]])
                    ppt = lambda c0, cn, cs=1: AP(
                        tensor=PP.tensor, offset=c0 * NT * F8,
                        ap=[[3 * NT * F8, 128], [cs * NT * F8, cn], [F8, NT],
                            [1, F8]])
                    mv1 = m1[:].rearrange("p (c t f) -> p c t f", c=3, t=NT)
                    mv2 = m2[:].rearrange("p (c t f) -> p c t f", c=3, t=NT)
                    nc.vector.tensor_tensor(mv1[:, 0:1], ppt(1, 1), wsh(2, 1),
                                            OP.mult)
                    nc.vector.tensor_tensor(mv1[:, 1:3], ppt(2, 2, -2),
                                            wsh(0, 2), OP.mult)
                    nc.vector.tensor_tensor(mv2[:, 0:1], ppt(2, 1), wsh(1, 1),
                                            OP.mult)
                    nc.vector.tensor_tensor(mv2[:, 1:3], ppt(0, 2), wsh(2, 2, -2),
                                            OP.mult)
                    nc.vector.tensor_sub(m1[:], m1[:], m2[:])
                    # csum over the 7 terms: pairwise bf16 adds (2x mode) beat
                    # a free-axis tensor_reduce (no fast mode); m2 is scratch
                    Ct = gtp.tile([128, 3 * F8], BF, name="Ct", tag="Ct")
                    c3 = Ct[:].rearrange("p (c f) -> p c f", c=3)
                    nc.vector.tensor_add(mv2[:, :, 0:3], mv1[:, :, 0:3],
                                         mv1[:, :, 3:6])
                    nc.vector.tensor_add(mv2[:, :, 3:4], mv2[:, :, 0:1],
                                         mv2[:, :, 1:2])
                    nc.vector.tensor_add(mv2[:, :, 4:5], mv2[:, :, 2:3],
                                         mv1[:, :, 6:7])
                    nc.vector.tensor_add(c3[:], mv2[:, :, 3], mv2[:, :, 4])
                    # phi' = S + (DT/2) C   (true phi = DT*phi')
                    nc.vector.scalar_tensor_tensor(c3[:], c3[:], DT / 2.0, s3[:],
                                                   OP.mult, OP.add)

                    # exp_small: u = |phi'|^2 -> Q8 group block (packed)
                    squ = gtp.tile([128, 3 * F8], FP, name="squ", tag="squ")
                    nc.scalar.activation(squ[:], Ct[:], AF.Square)
                    z3 = squ[:].rearrange("p (c f) -> p c f", c=3)
                    ut = gtp.tile([128, F8], FP, name="ut", tag="ut")
                    nc.vector.tensor_add(ut[:], z3[:, 0], z3[:, 1])
                    nc.vector.tensor_add(ut[:], ut[:], z3[:, 2])
                    qs = gtp.tile([128, F8], FP, name="qs_t", tag="qs")
                    gq = g * F8
                    q83 = Q8[:].rearrange("p (c f) -> p c f", c=4)
                    nc.scalar.activation(q83[:, 0, gq:gq + F8], ut[:], AF.Copy,
                                         bias=1.0, scale=-(DT * DT) / 8.0)
                    nc.scalar.activation(qs[:], ut[:], AF.Copy,
                                         bias=DT / 2.0, scale=-(DT ** 3) / 48.0)
                    qv_dst = AP(tensor=Q8.tensor, offset=256 + gq,
                                ap=[[4 * 256, 128], [256, 3], [1, F8]])
                    qsb = AP(tensor=qs.tensor, offset=0,
                             ap=[[F8, 128], [0, 3], [1, F8]])
                    nc.vector.tensor_tensor(qv_dst, qsb, c3[:], OP.mult)

                    # tree: Q8 -> Q16* (conj store)
                    qb = g * QBLK
                    qmul_packed(QPq, QPF, qb, Q8, 256, g * 128,
                                Q8, 256, g * 128 + 1, F16G, step=2,
                                conj_store=True, eng=nc.vector, eng2=nc.vector)
                    # level-4 Q: Q32* = qmul(Q16*_odd, Q16*_even)  (conj swap)
                    qmul_packed(QPq, QPF, qb + F16G, QPq, QPF, qb + 1,
                                QPq, QPF, qb, F32G, step=2,
                                eng=nc.gpsimd, eng2=nc.vector)

                    # -- velocity
                    HA = SEG + 17
                    HD = SEG + 16
                    gtt = gd.tile([128, 3 * R * SEG], BF, name="gtt", tag="gtt", bufs=1)
                    gt3 = gtt[:].rearrange("p (c r j) -> p c r j", c=3, r=R)
                    NB = 3 * R                     # 6 (c,r) blocks
                    FLT = NB * HA                  # 3174 flat scan length
                    aht = gd.tile([128, FLT + 1], BF, name="aht", tag="aht")
                    ah3 = aht[:, 0:FLT].rearrange("p (c r u) -> p c r u",
                                                  c=3, r=R)
                    for c in range(3):
                        nc.sync.dma_start(
                            out=gt3[:, c],
                            in_=gt[c, rows, :].rearrange("r (p j) -> p r j", j=SEG))
                        src = AP(tensor=a[:].tensor,
                                 offset=c * ROWS * N + g * R * N + 495,
                                 ap=[[SEG, 127], [N, R], [1, HA]])
                        nc.sync.dma_start(out=ah3[1:128, c], in_=src)
                        nc.sync.dma_start(out=ah3[0:1, c, :, 17:HA],
                                          in_=a[c, rows, 0:SEG])
                    nc.gpsimd.memset(ah3[0:1, :, :, 0:17], 0.0)
                    # a is host-prescaled by DT (s1 = DT*vs directly) and the
                    # 1/16 window mean folds into scan2's op1 -> every window
                    # op below is a plain tensor_sub (DVE 2x bf16 mode).
                    # NOTE: HW-measured — 12 per-(c,r) scans beat 2 flat
                    # [128,3174] scans by ~11us/rep despite identical sim
                    # cost; long serial scans underperform on HW. Keep 528.
                    s1 = gtp.tile([128, 3 * R * HD], BF, name="s1_t", tag="s1")
                    s13 = s1[:].rearrange("p (c r t) -> p c r t", c=3, r=R)
                    s2 = gtp.tile([128, 3 * R * HD], BF, name="s2_t", tag="s2")
                    s23 = s2[:].rearrange("p (c r t) -> p c r t", c=3, r=R)
                    c16b = AP(tensor=c16p.tensor, offset=0,
                              ap=[[1, 128], [0, HD]])
                    for c in range(3):
                        for r in range(R):
                            sv = nc.vector
                            sv.tensor_tensor_scan(
                                s13[:, c, r], ah3[:, c, r, 0:HD],
                                ah3[:, c, r, 1:HA], 0.0, OP.add, OP.add)
                            sv.tensor_tensor_scan(
                                s23[:, c, r], s13[:, c, r], c16b, 0.0,
                                OP.add, OP.mult)
                    t1 = gtp.tile([128, 3 * R * SEG], BF, name="t1_t", tag="t1")
                    t13 = t1[:].rearrange("p (c r t) -> p c r t", c=3, r=R)
                    for r in range(R):
                        err = s13[:, :, r, 0:SEG]
                        nc.vector.tensor_sub(t13[:, :, r], s23[:, :, r, 16:HD],
                                             s23[:, :, r, 0:SEG])
                        vsn = s23[:, :, r, 0:SEG]
                        nc.vector.tensor_sub(vsn, s13[:, :, r, 16:HD],
                                             t13[:, :, r])
                        nc.vector.tensor_sub(err, gt3[:, :, r], vsn)
                        col = 4 + g * 2 + r
                        sqd = gt3[:, :, r]
                        nc.scalar.activation(sqd, err, AF.Square,
                                             accum_out=st[:, col:col + 1])

                # ---------- merged D + log + huber (both groups, [128, 192]) ----
                if True:
                    QB2 = G * QBLK                 # 192: both groups contiguous
                    # D = Qstar x P for BOTH groups in one packed qmul
                    Dp = sp.tile([128, 4 * QB2], FP, name="Dp", tag="Dp")
                    qmul_packed(Dp, QB2, 0, QPq, QPF, 0, QPp, QPF, 0,
                                QB2, step=1, eng=nc.gpsimd, eng2=nc.vector)
                    d4 = Dp[:].rearrange("p (c f) -> p c f", c=4)

                    # log + huber on [128, 192]
                    l0 = [sp.tile([128, QB2], FP, name=f"lg{i}", tag=f"lg{i}")
                          for i in range(6)]
                    cm = sp.tile([128, QB2], mybir.dt.int32, name="cmask",
                                 tag="cmask")
                    nc.scalar.activation(l0[0][:], d4[:, 0], AF.Square)      # dw^2
                    nc.vector.tensor_scalar(l0[1][:], l0[0][:], 2.0, -1.0,
                                            OP.mult, OP.add)
                    nc.vector.tensor_scalar(l0[1][:], l0[1][:], 1.0 - 1e-7,
                                            -1.0 + 1e-7, OP.min, OP.max)     # c
                    nc.scalar.activation(l0[0][:], l0[1][:], AF.Square)
                    nc.scalar.activation(l0[2][:], l0[0][:], AF.Sqrt,
                                         bias=1.0, scale=-1.0)               # s
                    nc.scalar.activation(l0[3][:], l0[1][:], AF.Abs)         # |c|
                    nc.vector.tensor_tensor(l0[4][:], l0[2][:], l0[3][:], OP.min)
                    nc.vector.tensor_tensor(l0[5][:], l0[2][:], l0[3][:], OP.max)
                    nc.vector.reciprocal(l0[5][:], l0[5][:])
                    nc.vector.tensor_mul(l0[4][:], l0[4][:], l0[5][:])
                    nc.scalar.activation(l0[4][:], l0[4][:], AF.Arctan)      # atn
                    nc.vector.tensor_tensor(cm[:], l0[3][:], l0[2][:], OP.is_ge)
                    nc.scalar.activation(l0[5][:], l0[4][:], AF.Copy, scale=-1.0,
                                         bias=PI / 2.0)
                    nc.vector.copy_predicated(l0[5][:], cm[:], l0[4][:])     # th0
                    nc.vector.tensor_scalar(cm[:], l0[1][:], 0.0, None, OP.is_ge)
                    nc.scalar.activation(l0[3][:], l0[5][:], AF.Copy, scale=-1.0,
                                         bias=PI)
                    nc.vector.copy_predicated(l0[3][:], cm[:], l0[5][:])     # theta
                    nc.vector.reciprocal(l0[2][:], l0[2][:])
                    nc.vector.tensor_mul(l0[3][:], l0[3][:], l0[2][:])
                    nc.vector.scalar_tensor_tensor(l0[3][:], l0[3][:], 2.0,
                                                   d4[:, 0], OP.mult, OP.mult)
                    # huber (merged comps): rs = coef*dv
                    rsv = sp.tile([128, 3 * QB2], FP, name="rsv", tag="rsv")
                    r3 = rsv[:].rearrange("p (c f) -> p c f", c=3)
                    cfb2 = AP(tensor=l0[3].tensor, offset=0,
                              ap=[[QB2, 128], [0, 3], [1, QB2]])
                    nc.vector.tensor_tensor(r3[:], cfb2, d4[:, 1:4], OP.mult)
                    axv = sp.tile([128, 3 * QB2], FP, name="axv", tag="axv")
                    nc.scalar.activation(axv[:], rsv[:], AF.Abs, scale=1.0 / HUBER)
                    mv = sp.tile([128, 3 * QB2], FP, name="mv", tag="mv")
                    nc.vector.tensor_scalar_min(mv[:], axv[:], 1.0)
                    t5 = sp.tile([128, 3 * QB2], FP, name="t5", tag="t5")
                    nc.vector.scalar_tensor_tensor(t5[:], mv[:], -1.0, axv[:],
                                                   OP.mult, OP.add)
                    nc.vector.scalar_tensor_tensor(mv[:], mv[:], 0.5, mv[:],
                                                   OP.mult, OP.mult)
                    nc.gpsimd.tensor_add(t5[:], t5[:], mv[:])                # l
                    lt = t5[:].rearrange("p (c f) -> p c f", c=3)
                    lsum = sp.tile([128, QB2], FP, name="lsum", tag="lsum")
                    nc.gpsimd.tensor_add(lsum[:], lt[:, 0], lt[:, 1])
                    nc.gpsimd.tensor_add(lsum[:], lsum[:], lt[:, 2])
                    for base, width in ((0, F16G), (F16G, F32G),
                                        (QBLK, F16G), (QBLK + F16G, F32G)):
                        nc.vector.memset(
                            lsum[0:1, base:base + width].rearrange(
                                "p (row j) -> p row j",
                                j=width // R)[:, :, 0:N0], 0.0)
                    for col, base, width in ((1, 0, F16G), (2, F16G, F32G),
                                             (11, QBLK, F16G),
                                             (12, QBLK + F16G, F32G)):
                        nc.vector.tensor_reduce(st[:, col:col + 1],
                                                lsum[:, base:base + width],
                                                mybir.AxisListType.X, OP.add)

                nc.sync.dma_start(out=stats[:], in_=st[:])

    nc.compile()
    return nc


_NC = None


def _get_nc():
    global _NC
    if _NC is None:
        _NC = build_kernel()
    return _NC


def shard_inputs(w_hat, a_hat, xs, dv, vs_gt_norm):
    """Full inputs -> per-core input maps (planar component-major layout).

    w/a/gt are fed to the device as bf16 (host-side cast): halves DMA and
    unlocks the DVE 2x/4x perf modes; the loss tolerance (2e-2) dwarfs the
    bf16 rounding (see docstring)."""
    del dv  # unused by the reference computation
    import concourse.mybir as _mb
    bf = _mb.dt.np(_mb.dt.bfloat16)
    in_maps = []
    for core in range(CORES):
        rows = slice(core * ROWS, (core + 1) * ROWS)
        xsub = xs[rows, ::16]                      # [ROWS, M16, 3]
        xdev = xsub.reshape(ROWS, 128, M16 // 128, 3).transpose(3, 1, 0, 2)
        in_maps.append({
            "w": np.ascontiguousarray(w_hat[rows].transpose(2, 0, 1)).astype(bf),
            "a": (np.ascontiguousarray(a_hat[rows].transpose(2, 0, 1))
                  * np.float32(DT)).astype(bf),
            "gt": np.ascontiguousarray(vs_gt_norm[rows].transpose(2, 0, 1)).astype(bf),
            "xs": np.ascontiguousarray(xdev.reshape(3, 128, 128)),
        })
    return in_maps


def combine_stats(stats_list):
    """Per-core [128,16] partials -> final scalar loss (fp64 host combine)."""
    s = np.sum([st.astype(np.float64) for st in stats_list], axis=(0, 1))
    sq_total = float(np.sum(s[4:10]))
    l16 = float(s[1] + s[11])
    l32 = float(s[2] + s[12])
    acc = sq_total / (B * N * 3)
    g16 = W_LOSS * HUBER * HUBER * l16 / (B * (M16 - N0) * 3)
    g32 = W_LOSS * HUBER * HUBER * l32 / (B * (M32 - N0) * 3) / 2.0
    return np.float32(g16 + g32 + acc)


def kernel(**inputs):
    nc = _get_nc()
    in_maps = shard_inputs(**inputs)
    res = run_bass_kernel_spmd(nc, in_maps, list(range(CORES)))
    return combine_stats([r["stats"] for r in res.results])

